# revision 1
# baseline (speedup 1.0000x reference)
"""MultiHeadedAttention Trainium2 Bass kernel.

Reference (per batch element b, full shapes B=8, S=1024, D=512, H=8, DK=64):
    Q = x_q @ Wq + bq ; K = x_k @ Wk + bk ; V = x_v @ Wv + bv   (per-head split)
    S = Q K^T / sqrt(DK);  S masked where mask==0 -> -inf
    P = softmax(S); P zeroed where mask==0
    Y = (P V, heads concat) @ Wo + bo

Sharding: pure data parallel over batch — core c computes batch element c.
No collectives. Host transposes x inputs so the kernel needs no on-chip
input transposes, and precomputes the additive exp-space mask bias.

Per-core layout (f32 in HBM; matmuls run as f32r, PSUM accumulates f32):
  xT        [in=512, S]  (host-transposed; DMA interleaved with weights so
                          the first projection starts after ~3MB, not 12MB)
  QT, KT    [feat, S]   psum[out128, q512] += Wq[in128, out128].T @ xT[in128, q512]
  V natural [S, feat]   psum[row128, f512] += xT_v[in128, row128].T @ Wv[in128, f512]
                        stored interleaved as v_aug[row128, head, 65] with a
                        ones column per head (softmax denominator for free);
                        one strided DVE copy per row tile
  S^T       [k128, q512] = KT_h[d64, k128].T @ QT_h[d64, q512]
                        head pairs packed into PE row groups 0/64 via
                        tile_position -> both matmuls run concurrently
  P^T       = Exp(S^T/8 + maskbias_k)      (ACT, one call per [128,1024])
  (PV)^T+den[65, q512]  += v_aug_h[k128, 65].T @ P^T[k128, q512]  (row 64 = denom)
  norm      at_pair[t][h%2*64 :+64, q] = (PV)^T * bcast(1/denom)
            (DVE cross-base-partition write packs head pairs -> K=128 below)
  Y natural [q128, 512] += at_pair[t][:, q128].T @ Wo[feat128, out512] (+ bo)
"""

import numpy as np

B, S, D, H = 8, 1024, 512, 8
DK = D // H  # 64
P = 128
KI = D // P  # 4 in-feature tiles
RT = S // P  # 8 row tiles
QC = S // 512  # 2 q chunks of 512
HP = H // 2  # 4 head pairs
MASK_NEG = -30000.0  # exp(-30000) == 0.0 in f32

_CACHED = {}


def _build_nc(loop_reps=None):
    import concourse.mybir as mybir
    import concourse.tile as tile
    from concourse import bacc

    f32 = mybir.dt.float32
    f32r = mybir.dt.float32r
    EXP = mybir.ActivationFunctionType.Exp
    ISCALE = 1.0 / float(np.sqrt(DK))

    nc = bacc.Bacc("TRN2")

    xqT_d = nc.dram_tensor("xqT", (KI, P, S), f32r, kind="ExternalInput")
    xkT_d = nc.dram_tensor("xkT", (KI, P, S), f32r, kind="ExternalInput")
    xvT_d = nc.dram_tensor("xvT", (KI, P, S), f32r, kind="ExternalInput")
    maskb_d = nc.dram_tensor("maskb", (P, RT), f32, kind="ExternalInput")
    wq_d = nc.dram_tensor("wq", (KI, P, D), f32r, kind="ExternalInput")
    wk_d = nc.dram_tensor("wk", (KI, P, D), f32r, kind="ExternalInput")
    wv_d = nc.dram_tensor("wv", (KI, P, D), f32r, kind="ExternalInput")
    wo_d = nc.dram_tensor("wo", (KI, P, D), f32r, kind="ExternalInput")
    bq_d = nc.dram_tensor("bq", (P, KI), f32, kind="ExternalInput")
    bk_d = nc.dram_tensor("bk", (P, KI), f32, kind="ExternalInput")
    bv_d = nc.dram_tensor("bv", (1, D), f32r, kind="ExternalInput")
    bo_d = nc.dram_tensor("bo", (1, D), f32r, kind="ExternalInput")
    y_d = nc.dram_tensor("y", (RT, P, D), f32, kind="ExternalOutput")

    with tile.TileContext(nc) as tc, nc.allow_low_precision(
        reason="f32r is fp32-width storage; matmul accumulation stays fp32 in PSUM"
    ):
        from contextlib import ExitStack

        def emit():
            with ExitStack() as ctx:
                const = ctx.enter_context(tc.tile_pool(name="const", bufs=1))
                persist = ctx.enter_context(tc.tile_pool(name="persist", bufs=1))

                wq = [const.tile([P, D], f32r, name=f"wq{i}", tag=f"wq{i}") for i in range(KI)]
                wk = [const.tile([P, D], f32r, name=f"wk{i}", tag=f"wk{i}") for i in range(KI)]
                wv = [const.tile([P, D], f32r, name=f"wv{i}", tag=f"wv{i}") for i in range(KI)]
                wo = [const.tile([P, D], f32r, name=f"wo{i}", tag=f"wo{i}") for i in range(KI)]
                bq_t = const.tile([P, KI], f32, name="bq_t", tag="bq")
                bk_t = const.tile([P, KI], f32, name="bk_t", tag="bk")
                bv_t = const.tile([1, D], f32r, name="bv_t", tag="bv")
                bo_t = const.tile([1, D], f32r, name="bo_t", tag="bo")
                maskb = const.tile([P, RT], f32, name="maskb", tag="maskb")
                ones_t = const.tile([P, P], f32r, name="ones_t", tag="ones")
                nc.vector.memset(ones_t[:].bitcast(f32), 1.0)

                # persistent intermediates
                qt = [persist.tile([P, S], f32r, name=f"qt{i}", tag=f"qt{i}") for i in range(KI)]
                kt_ = [persist.tile([P, S], f32r, name=f"kt{i}", tag=f"kt{i}") for i in range(KI)]
                v_aug = [persist.tile([P, H, DK + 1], f32r, name=f"va{i}", tag=f"va{i}") for i in range(RT)]
                # head-pair attention outputs: pair t rows 0:64 = head 2t,
                # rows 64:128 = head 2t+1 => feature rows 128t..128t+127
                at = [persist.tile([P, S], f32r, name=f"at{i}", tag=f"at{i}") for i in range(HP)]

                with ExitStack() as actx:
                    xt_pool = actx.enter_context(tc.tile_pool(name="xt", bufs=1))
                    psA = actx.enter_context(
                        tc.tile_pool(name="psA", bufs=4, space="PSUM")
                    )
                    xqT = [xt_pool.tile([P, S], f32r, name=f"xq{i}", tag=f"xq{i}") for i in range(KI)]
                    xkT = [xt_pool.tile([P, S], f32r, name=f"xk{i}", tag=f"xk{i}") for i in range(KI)]
                    xvT = [xt_pool.tile([P, S], f32r, name=f"xv{i}", tag=f"xv{i}") for i in range(KI)]

                    # DMA in consumption order (queue is FIFO): q-path first
                    # so the first projection can start after ~3MB.
                    # Two HWDGE queues: q-path then v-path on sync's queue,
                    # k-path then output weights on ScalarE's queue (descriptor
                    # issue only; ACT is idle during the load). QT and KT
                    # stream in parallel so the first S^T starts ~2x sooner.
                    for i in range(KI):
                        nc.sync.dma_start(wq[i][:], wq_d[i])
                        nc.sync.dma_start(xqT[i][:], xqT_d[i])
                        nc.scalar.dma_start(wk[i][:], wk_d[i])
                        nc.scalar.dma_start(xkT[i][:], xkT_d[i])
                    nc.sync.dma_start(bq_t[:], bq_d[:])
                    nc.scalar.dma_start(bk_t[:], bk_d[:])
                    nc.scalar.dma_start(maskb[:], maskb_d[:])
                    for i in range(KI):
                        nc.sync.dma_start(wv[i][:], wv_d[i])
                        nc.sync.dma_start(xvT[i][:], xvT_d[i])
                    nc.sync.dma_start(bv_t[:], bv_d[:])
                    for i in range(KI):
                        nc.scalar.dma_start(wo[i][:], wo_d[i])
                    nc.scalar.dma_start(bo_t[:], bo_d[:])

                    # QT / KT projections
                    for w, x, bias, dst in ((wq, xqT, bq_t, qt), (wk, xkT, bk_t, kt_)):
                        for o in range(KI):
                            for qc in range(QC):
                                ps = psA.tile([P, 512], f32, name="psA", tag="psA")
                                for ki in range(KI):
                                    nc.tensor.matmul(
                                        ps[:],
                                        w[ki][:, o * P : (o + 1) * P],
                                        x[ki][:, qc * 512 : (qc + 1) * 512],
                                        start=(ki == 0),
                                        stop=(ki == KI - 1),
                                    )
                                nc.vector.tensor_scalar_add(
                                    dst[o][:, qc * 512 : (qc + 1) * 512],
                                    ps[:],
                                    bias[:, o : o + 1],
                                )

                    # V natural -> v_aug (interleaved heads + ones columns)
                    for rt in range(RT):
                        ps = psA.tile([P, 512], f32, name="psA", tag="psA")
                        for ki in range(KI):
                            nc.tensor.matmul(
                                ps[:],
                                xvT[ki][:, rt * P : (rt + 1) * P],
                                wv[ki][:],
                                start=(ki == 0),
                                stop=False,
                            )
                        nc.tensor.matmul(
                            ps[:],
                            ones_t[0:1, 0:P],
                            bv_t[0:1, :],
                            start=False,
                            stop=True,
                        )
                        nc.vector.tensor_copy(
                            v_aug[rt][:, :, 0:DK],
                            ps[:].rearrange("p (h d) -> p h d", h=H),
                        )
                        nc.vector.memset(
                            v_aug[rt][:, :, DK : DK + 1].bitcast(f32), 1.0
                        )

                # --- attention, one head pair at a time ---
                with ExitStack() as bctx:
                    pt_pool = bctx.enter_context(tc.tile_pool(name="pt", bufs=21))
                    rec_pool = bctx.enter_context(tc.tile_pool(name="rec", bufs=4))
                    at_ps = bctx.enter_context(
                        tc.tile_pool(name="spsum", bufs=2, space="PSUM")
                    )
                    ov_ps = bctx.enter_context(
                        tc.tile_pool(name="opsum", bufs=3, space="PSUM")
                    )
                    rb_ps = bctx.enter_context(
                        tc.tile_pool(name="rbpsum", bufs=1, space="PSUM")
                    )

                    for t in range(HP):
                        pts = [
                            [pt_pool.tile([P, S], f32r, name="pt", tag="pt") for _ in range(RT)]
                            for _ in range(2)
                        ]
                        # sub 0's PV chains consume pt tiles in lockstep with
                        # the exp stream so half the pool frees at pair end
                        # (the next pair's exps then aren't slot-starved).
                        ops00 = ov_ps.tile([P, 512], f32, name="ops", tag="ops")
                        ops01 = ov_ps.tile([P, 512], f32, name="ops", tag="ops")
                        for kt in range(RT):
                            for sub in range(2):
                                off = sub * DK
                                sps = at_ps.tile([P, S], f32, name="sps", tag="sps")
                                for qc in range(QC):
                                    nc.tensor.matmul(
                                        sps[:, qc * 512 : (qc + 1) * 512],
                                        kt_[t][off : off + DK, kt * P : (kt + 1) * P],
                                        qt[t][off : off + DK, qc * 512 : (qc + 1) * 512],
                                        start=True,
                                        stop=True,
                                        tile_position=(off, 0),
                                    )
                                nc.scalar.activation(
                                    pts[sub][kt][:],
                                    sps[:],
                                    EXP,
                                    bias=maskb[:, kt : kt + 1],
                                    scale=ISCALE,
                                )
                            for qc, ops in ((0, ops00), (1, ops01)):
                                nc.tensor.matmul(
                                    ops[0 : DK + 1, :],
                                    v_aug[kt][:, 2 * t, 0 : DK + 1],
                                    pts[0][kt][:, qc * 512 : (qc + 1) * 512],
                                    start=(kt == 0),
                                    stop=(kt == RT - 1),
                                )
                        for sub in range(2):
                            h = 2 * t + sub
                            off = sub * DK
                            for qc in range(QC):
                                if sub == 0:
                                    ops = ops00 if qc == 0 else ops01
                                else:
                                    ops = ov_ps.tile(
                                        [P, 512], f32, name="ops", tag="ops"
                                    )
                                    for kt in range(RT):
                                        nc.tensor.matmul(
                                            ops[0 : DK + 1, :],
                                            v_aug[kt][:, h, 0 : DK + 1],
                                            pts[sub][kt][:, qc * 512 : (qc + 1) * 512],
                                            start=(kt == 0),
                                            stop=(kt == RT - 1),
                                        )
                                rec = rec_pool.tile(
                                    [DK + 1, 512], f32r, name="rec", tag="rec"
                                )
                                nc.vector.reciprocal(
                                    rec[DK : DK + 1, :], ops[DK : DK + 1, :]
                                )
                                rb = rb_ps.tile([P, 512], f32, name="rb", tag="rb")
                                nc.tensor.matmul(
                                    rb[0:DK, :],
                                    ones_t[DK : DK + 1, 0:DK],
                                    rec[DK : DK + 1, :],
                                    start=True,
                                    stop=True,
                                )
                                # DVE has one PSUM read port: stage broadcast
                                # reciprocal in SBUF, then multiply (write may
                                # shift base partition by 64 for odd heads).
                                rbs = rec_pool.tile(
                                    [DK, 512], f32, name="rbs", tag="rbs"
                                )
                                nc.vector.tensor_copy(rbs[:], rb[0:DK, :])
                                nc.vector.tensor_mul(
                                    at[t][off : off + DK, qc * 512 : (qc + 1) * 512],
                                    ops[0:DK, :],
                                    rbs[:],
                                )

                # --- output projection: contraction K=128 over head pairs ---
                with ExitStack() as cctx:
                    y_pool = cctx.enter_context(tc.tile_pool(name="y", bufs=3))
                    y_ps = cctx.enter_context(
                        tc.tile_pool(name="ypsum", bufs=2, space="PSUM")
                    )
                    for rt in range(RT):
                        yps = y_ps.tile([P, D], f32, name="yps", tag="yps")
                        for t in range(HP):
                            nc.tensor.matmul(
                                yps[:],
                                at[t][:, rt * P : (rt + 1) * P],
                                wo[t][:],
                                start=(t == 0),
                                stop=False,
                            )
                        nc.tensor.matmul(
                            yps[:],
                            ones_t[0:1, 0:P],
                            bo_t[0:1, :],
                            start=False,
                            stop=True,
                        )
                        yt = y_pool.tile([P, D], f32, name="yt", tag="yt")
                        nc.vector.tensor_copy(yt[:], yps[:])
                        nc.sync.dma_start(y_d[rt], yt[:])

        if loop_reps is None:
            emit()
        else:
            # benchmark variant: repeat the whole body on-device
            ET = mybir.EngineType
            with tc.For_i(
                0,
                loop_reps,
                1,
                hint_engines=(ET.PE, ET.Activation, ET.DVE, ET.SP, ET.Pool),
            ):
                emit()

    nc.compile()
    return nc


def get_nc(loop_reps=None):
    key = ("nc", loop_reps)
    if key not in _CACHED:
        _CACHED[key] = _build_nc(loop_reps)
    return _CACHED[key]


def make_in_maps(query, key, value, mask, Wq, bq, Wk, bk, Wv, bv, Wo, bo):
    """Shard full inputs into per-core input maps (host-side numpy)."""
    f = np.float32
    query = np.asarray(query, f)
    key = np.asarray(key, f)
    value = np.asarray(value, f)
    mask = np.asarray(mask)

    def wtiles(W):
        return np.ascontiguousarray(np.asarray(W, f).reshape(KI, P, D))

    wq_t, wk_t, wv_t, wo_t = wtiles(Wq), wtiles(Wk), wtiles(Wv), wtiles(Wo)
    bq_t = np.ascontiguousarray(np.asarray(bq, f).reshape(KI, P).T)
    bk_t = np.ascontiguousarray(np.asarray(bk, f).reshape(KI, P).T)
    bv_t = np.ascontiguousarray(np.asarray(bv, f).reshape(1, D))
    bo_t = np.ascontiguousarray(np.asarray(bo, f).reshape(1, D))

    in_maps = []
    for c in range(B):
        xqT = np.ascontiguousarray(query[c].T).reshape(KI, P, S)
        xkT = np.ascontiguousarray(key[c].T).reshape(KI, P, S)
        xvT = np.ascontiguousarray(value[c].T).reshape(KI, P, S)
        mb = np.where(mask[c, 0] == 0, f(MASK_NEG), f(0.0)).astype(f)
        mb = np.ascontiguousarray(mb.reshape(RT, P).T)
        in_maps.append(
            {
                "xqT": xqT,
                "xkT": xkT,
                "xvT": xvT,
                "maskb": mb,
                "wq": wq_t,
                "wk": wk_t,
                "wv": wv_t,
                "wo": wo_t,
                "bq": bq_t,
                "bk": bk_t,
                "bv": bv_t,
                "bo": bo_t,
            }
        )
    return in_maps


def kernel(**inputs):
    from concourse.bass_utils import run_bass_kernel_spmd

    nc = get_nc()
    in_maps = make_in_maps(**inputs)
    res = run_bass_kernel_spmd(nc, in_maps, core_ids=list(range(B)))
    out = np.stack([res.results[c]["y"].reshape(S, D) for c in range(B)])
    return out.astype(np.float32)



# revision 3
# speedup vs baseline: 1.4099x; 1.4099x over previous
"""MultiHeadedAttention Trainium2 Bass kernel.

Reference (per batch element b, full shapes B=8, S=1024, D=512, H=8, DK=64):
    Q = x_q @ Wq + bq ; K = x_k @ Wk + bk ; V = x_v @ Wv + bv   (per-head split)
    S = Q K^T / sqrt(DK);  S masked where mask==0 -> -inf
    P = softmax(S); P zeroed where mask==0
    Y = (P V, heads concat) @ Wo + bo

Sharding: pure data parallel over batch — core c computes batch element c.
No collectives. Host transposes x inputs so the kernel needs no on-chip
input transposes, precomputes the additive exp-space mask bias, and
rounds matmul operands to bf16 (PSUM accumulation stays fp32; measured
bf16 matmul streams ~15% faster than f32r on HW and halves HBM+SBUF).

Engine assignment (HW shows ~1us fixed cost per DVE op, so DVE glue is
offloaded): ACT does exp + all PSUM->SBUF moves (Copy and Exp live in
the same activation table, so no table swaps); Pool (gpsimd) broadcasts
the softmax reciprocal across partitions; DVE keeps only reciprocal and
the normalize multiply.

Per-core layout (bf16 matmul operands; PSUM accumulates f32):
  xT        [in=512, S]  (host-transposed; DMA interleaved with weights so
                          the first projection starts after ~1.5MB)
  QT, KT    [feat, S]   psum[out128, q512] += Wq[in128, out128].T @ xT[in128, q512]
                        bias folded into the ACT Copy that moves PSUM->SBUF
  V natural [S, feat]   psum[row128, f512] += xT_v[in128, row128].T @ Wv[in128, f512]
                        (+ ones-row x bv outer product), stored interleaved as
                        v_aug[row128, head, 65] with a ones column per head
                        (softmax denominator for free); one ACT copy per row tile
  S^T       [k128, q512] = KT_h[d64, k128].T @ QT_h[d64, q512]
                        head pairs packed into PE row groups 0/64 via
                        tile_position
  P^T       = Exp(S^T/8 + maskbias_k)      (ACT, one call per [128,1024])
  (PV)^T+den[65, q512]  += v_aug_h[k128, 65].T @ P^T[k128, q512]  (row 64 = denom)
  norm      rec = 1/den (DVE), rbs = bcast rec over 64 partitions (Pool),
            at_pair[t][h%2*64 :+64, q] = (PV)^T * rbs (DVE; write may shift
            base partition by 64 for odd heads -> K=128 below)
  Y natural [q128, 512] += at_pair[t][:, q128].T @ Wo[feat128, out512] (+ bo)
"""

import numpy as np

B, S, D, H = 8, 1024, 512, 8
DK = D // H  # 64
P = 128
KI = D // P  # 4 in-feature tiles
RT = S // P  # 8 row tiles
QC = S // 512  # 2 q chunks of 512
HP = H // 2  # 4 head pairs
MASK_NEG = -30000.0  # exp(-30000) == 0.0 in f32

_CACHED = {}


def _build_nc(loop_reps=None):
    import concourse.mybir as mybir
    import concourse.tile as tile
    from concourse import bacc

    f32 = mybir.dt.float32
    bf16 = mybir.dt.bfloat16
    EXP = mybir.ActivationFunctionType.Exp
    CPY = mybir.ActivationFunctionType.Copy
    IDN = mybir.ActivationFunctionType.Identity
    ISCALE = 1.0 / float(np.sqrt(DK))

    nc = bacc.Bacc("TRN2")

    xqT_d = nc.dram_tensor("xqT", (KI, P, S), bf16, kind="ExternalInput")
    xkT_d = nc.dram_tensor("xkT", (KI, P, S), bf16, kind="ExternalInput")
    xvT_d = nc.dram_tensor("xvT", (KI, P, S), bf16, kind="ExternalInput")
    maskb_d = nc.dram_tensor("maskb", (P, RT), f32, kind="ExternalInput")
    wq_d = nc.dram_tensor("wq", (KI, P, D), bf16, kind="ExternalInput")
    wk_d = nc.dram_tensor("wk", (KI, P, D), bf16, kind="ExternalInput")
    wv_d = nc.dram_tensor("wv", (KI, P, D), bf16, kind="ExternalInput")
    wo_d = nc.dram_tensor("wo", (KI, P, D), bf16, kind="ExternalInput")
    bq_d = nc.dram_tensor("bq", (P, KI), f32, kind="ExternalInput")
    bk_d = nc.dram_tensor("bk", (P, KI), f32, kind="ExternalInput")
    bv_d = nc.dram_tensor("bv", (1, D), bf16, kind="ExternalInput")
    bo_d = nc.dram_tensor("bo", (1, D), bf16, kind="ExternalInput")
    y_d = nc.dram_tensor("y", (RT, P, D), f32, kind="ExternalOutput")

    with tile.TileContext(nc) as tc, nc.allow_low_precision(
        reason="bf16 matmul operands; accumulation stays fp32 in PSUM"
    ):
        from contextlib import ExitStack

        def emit():
            with ExitStack() as ctx:
                const = ctx.enter_context(tc.tile_pool(name="const", bufs=1))
                persist = ctx.enter_context(tc.tile_pool(name="persist", bufs=1))

                wq = [const.tile([P, D], bf16, name=f"wq{i}", tag=f"wq{i}") for i in range(KI)]
                wk = [const.tile([P, D], bf16, name=f"wk{i}", tag=f"wk{i}") for i in range(KI)]
                wv = [const.tile([P, D], bf16, name=f"wv{i}", tag=f"wv{i}") for i in range(KI)]
                wo = [const.tile([P, D], bf16, name=f"wo{i}", tag=f"wo{i}") for i in range(KI)]
                bq_t = const.tile([P, KI], f32, name="bq_t", tag="bq")
                bk_t = const.tile([P, KI], f32, name="bk_t", tag="bk")
                bv_t = const.tile([1, D], bf16, name="bv_t", tag="bv")
                bo_t = const.tile([1, D], bf16, name="bo_t", tag="bo")
                maskb = const.tile([P, RT], f32, name="maskb", tag="maskb")
                ones_t = const.tile([1, P], bf16, name="ones_t", tag="ones")
                nc.gpsimd.memset(ones_t[:], 1.0)

                # persistent intermediates
                qt = [persist.tile([P, S], bf16, name=f"qt{i}", tag=f"qt{i}") for i in range(KI)]
                kt_ = [persist.tile([P, S], bf16, name=f"kt{i}", tag=f"kt{i}") for i in range(KI)]
                v_aug = [persist.tile([P, H, DK + 1], bf16, name=f"va{i}", tag=f"va{i}") for i in range(RT)]
                # head-pair attention outputs: pair t rows 0:64 = head 2t,
                # rows 64:128 = head 2t+1 => feature rows 128t..128t+127
                at = [persist.tile([P, S], bf16, name=f"at{i}", tag=f"at{i}") for i in range(HP)]

                with ExitStack() as actx:
                    xt_pool = actx.enter_context(tc.tile_pool(name="xt", bufs=1))
                    psA = actx.enter_context(
                        tc.tile_pool(name="psA", bufs=4, space="PSUM")
                    )
                    xqT = [xt_pool.tile([P, S], bf16, name=f"xq{i}", tag=f"xq{i}") for i in range(KI)]
                    xkT = [xt_pool.tile([P, S], bf16, name=f"xk{i}", tag=f"xk{i}") for i in range(KI)]
                    xvT = [xt_pool.tile([P, S], bf16, name=f"xv{i}", tag=f"xv{i}") for i in range(KI)]

                    # DMA in consumption order (queue is FIFO): q-path first
                    # so the first projection can start after ~1.5MB.
                    # Two HWDGE queues: q-path, v-path and output weights on
                    # sync's queue, k-path on ScalarE's queue (descriptor
                    # issue only, before ACT's first exp of the iteration).
                    # QT and KT stream in parallel so the first S^T starts
                    # ~2x sooner.
                    for i in range(KI):
                        nc.sync.dma_start(wq[i][:], wq_d[i])
                        nc.sync.dma_start(xqT[i][:], xqT_d[i])
                        nc.scalar.dma_start(wk[i][:], wk_d[i])
                        nc.scalar.dma_start(xkT[i][:], xkT_d[i])
                    nc.sync.dma_start(bq_t[:], bq_d[:])
                    nc.scalar.dma_start(bk_t[:], bk_d[:])
                    nc.scalar.dma_start(maskb[:], maskb_d[:])
                    for i in range(KI):
                        nc.sync.dma_start(wv[i][:], wv_d[i])
                        nc.sync.dma_start(xvT[i][:], xvT_d[i])
                    nc.sync.dma_start(bv_t[:], bv_d[:])
                    for i in range(KI):
                        nc.sync.dma_start(wo[i][:], wo_d[i])
                    nc.sync.dma_start(bo_t[:], bo_d[:])

                    # QT / KT projections; ACT moves PSUM->SBUF, adding the
                    # per-partition bias during the copy.
                    for w, x, bias, dst in ((wq, xqT, bq_t, qt), (wk, xkT, bk_t, kt_)):
                        for o in range(KI):
                            for qc in range(QC):
                                ps = psA.tile([P, 512], f32, name="psA", tag="psA")
                                for ki in range(KI):
                                    nc.tensor.matmul(
                                        ps[:],
                                        w[ki][:, o * P : (o + 1) * P],
                                        x[ki][:, qc * 512 : (qc + 1) * 512],
                                        start=(ki == 0),
                                        stop=(ki == KI - 1),
                                    )
                                nc.scalar.activation(
                                    dst[o][:, qc * 512 : (qc + 1) * 512],
                                    ps[:],
                                    IDN,
                                    bias=bias[:, o : o + 1],
                                )

                    # V natural -> v_aug (interleaved heads + ones columns)
                    for rt in range(RT):
                        ps = psA.tile([P, 512], f32, name="psA", tag="psA")
                        for ki in range(KI):
                            nc.tensor.matmul(
                                ps[:],
                                xvT[ki][:, rt * P : (rt + 1) * P],
                                wv[ki][:],
                                start=(ki == 0),
                                stop=False,
                            )
                        nc.tensor.matmul(
                            ps[:],
                            ones_t[0:1, 0:P],
                            bv_t[0:1, :],
                            start=False,
                            stop=True,
                        )
                        nc.scalar.activation(
                            v_aug[rt][:, :, 0:DK],
                            ps[:].rearrange("p (h d) -> p h d", h=H),
                            CPY,
                        )
                        nc.gpsimd.memset(v_aug[rt][:, :, DK : DK + 1], 1.0)

                # --- attention, one head pair at a time ---
                with ExitStack() as bctx:
                    pt_pool = bctx.enter_context(tc.tile_pool(name="pt", bufs=24))
                    rec_pool = bctx.enter_context(tc.tile_pool(name="rec", bufs=4))
                    rbs_pool = bctx.enter_context(tc.tile_pool(name="rbs", bufs=4))
                    at_ps = bctx.enter_context(
                        tc.tile_pool(name="spsum", bufs=2, space="PSUM")
                    )
                    ov_ps = bctx.enter_context(
                        tc.tile_pool(name="opsum", bufs=3, space="PSUM")
                    )

                    for t in range(HP):
                        pts = [
                            [pt_pool.tile([P, S], bf16, name="pt", tag="pt") for _ in range(RT)]
                            for _ in range(2)
                        ]
                        # sub 0's PV chains consume pt tiles in lockstep with
                        # the exp stream so half the pool frees at pair end
                        # (the next pair's exps then aren't slot-starved).
                        ops00 = ov_ps.tile([P, 512], f32, name="ops", tag="ops")
                        ops01 = ov_ps.tile([P, 512], f32, name="ops", tag="ops")
                        for kt in range(RT):
                            for sub in range(2):
                                off = sub * DK
                                sps = at_ps.tile([P, S], f32, name="sps", tag="sps")
                                for qc in range(QC):
                                    nc.tensor.matmul(
                                        sps[:, qc * 512 : (qc + 1) * 512],
                                        kt_[t][off : off + DK, kt * P : (kt + 1) * P],
                                        qt[t][off : off + DK, qc * 512 : (qc + 1) * 512],
                                        start=True,
                                        stop=True,
                                        tile_position=(off, 0),
                                    )
                                nc.scalar.activation(
                                    pts[sub][kt][:],
                                    sps[:],
                                    EXP,
                                    bias=maskb[:, kt : kt + 1],
                                    scale=ISCALE,
                                )
                            for qc, ops in ((0, ops00), (1, ops01)):
                                nc.tensor.matmul(
                                    ops[0 : DK + 1, :],
                                    v_aug[kt][:, 2 * t, 0 : DK + 1],
                                    pts[0][kt][:, qc * 512 : (qc + 1) * 512],
                                    start=(kt == 0),
                                    stop=(kt == RT - 1),
                                )
                        for sub in range(2):
                            h = 2 * t + sub
                            off = sub * DK
                            for qc in range(QC):
                                if sub == 0:
                                    ops = ops00 if qc == 0 else ops01
                                else:
                                    ops = ov_ps.tile(
                                        [P, 512], f32, name="ops", tag="ops"
                                    )
                                    for kt in range(RT):
                                        nc.tensor.matmul(
                                            ops[0 : DK + 1, :],
                                            v_aug[kt][:, h, 0 : DK + 1],
                                            pts[sub][kt][:, qc * 512 : (qc + 1) * 512],
                                            start=(kt == 0),
                                            stop=(kt == RT - 1),
                                        )
                                rec = rec_pool.tile([1, 512], f32, name="rec", tag="rec")
                                nc.vector.reciprocal(
                                    rec[0:1, :], ops[DK : DK + 1, :]
                                )
                                rbs = rbs_pool.tile(
                                    [DK, 512], f32, name="rbs", tag="rbs"
                                )
                                nc.gpsimd.partition_broadcast(rbs[:], rec[0:1, :])
                                nc.vector.tensor_mul(
                                    at[t][off : off + DK, qc * 512 : (qc + 1) * 512],
                                    ops[0:DK, :],
                                    rbs[:],
                                )

                # --- output projection: contraction K=128 over head pairs ---
                with ExitStack() as cctx:
                    y_pool = cctx.enter_context(tc.tile_pool(name="y", bufs=3))
                    y_ps = cctx.enter_context(
                        tc.tile_pool(name="ypsum", bufs=2, space="PSUM")
                    )
                    for rt in range(RT):
                        yps = y_ps.tile([P, D], f32, name="yps", tag="yps")
                        for t in range(HP):
                            nc.tensor.matmul(
                                yps[:],
                                at[t][:, rt * P : (rt + 1) * P],
                                wo[t][:],
                                start=(t == 0),
                                stop=False,
                            )
                        nc.tensor.matmul(
                            yps[:],
                            ones_t[0:1, 0:P],
                            bo_t[0:1, :],
                            start=False,
                            stop=True,
                        )
                        yt = y_pool.tile([P, D], f32, name="yt", tag="yt")
                        nc.scalar.activation(yt[:], yps[:], CPY)
                        nc.sync.dma_start(y_d[rt], yt[:])

        if loop_reps is None:
            emit()
        else:
            # benchmark variant: repeat the whole body on-device
            ET = mybir.EngineType
            with tc.For_i(
                0,
                loop_reps,
                1,
                hint_engines=(ET.PE, ET.Activation, ET.DVE, ET.SP, ET.Pool),
            ):
                emit()

    nc.compile()
    return nc


def get_nc(loop_reps=None):
    key = ("nc", loop_reps)
    if key not in _CACHED:
        _CACHED[key] = _build_nc(loop_reps)
    return _CACHED[key]


def make_in_maps(query, key, value, mask, Wq, bq, Wk, bk, Wv, bv, Wo, bo):
    """Shard full inputs into per-core input maps (host-side numpy)."""
    import ml_dtypes

    f = np.float32
    bf = ml_dtypes.bfloat16
    query = np.asarray(query, f)
    key = np.asarray(key, f)
    value = np.asarray(value, f)
    mask = np.asarray(mask)

    def wtiles(W):
        return np.ascontiguousarray(
            np.asarray(W, f).reshape(KI, P, D).astype(bf)
        )

    wq_t, wk_t, wv_t, wo_t = wtiles(Wq), wtiles(Wk), wtiles(Wv), wtiles(Wo)
    bq_t = np.ascontiguousarray(np.asarray(bq, f).reshape(KI, P).T)
    bk_t = np.ascontiguousarray(np.asarray(bk, f).reshape(KI, P).T)
    bv_t = np.ascontiguousarray(np.asarray(bv, f).reshape(1, D).astype(bf))
    bo_t = np.ascontiguousarray(np.asarray(bo, f).reshape(1, D).astype(bf))

    in_maps = []
    for c in range(B):
        xqT = np.ascontiguousarray(query[c].T).reshape(KI, P, S).astype(bf)
        xkT = np.ascontiguousarray(key[c].T).reshape(KI, P, S).astype(bf)
        xvT = np.ascontiguousarray(value[c].T).reshape(KI, P, S).astype(bf)
        mb = np.where(mask[c, 0] == 0, f(MASK_NEG), f(0.0)).astype(f)
        mb = np.ascontiguousarray(mb.reshape(RT, P).T)
        in_maps.append(
            {
                "xqT": xqT,
                "xkT": xkT,
                "xvT": xvT,
                "maskb": mb,
                "wq": wq_t,
                "wk": wk_t,
                "wv": wv_t,
                "wo": wo_t,
                "bq": bq_t,
                "bk": bk_t,
                "bv": bv_t,
                "bo": bo_t,
            }
        )
    return in_maps


def kernel(**inputs):
    from concourse.bass_utils import run_bass_kernel_spmd

    nc = get_nc()
    in_maps = make_in_maps(**inputs)
    res = run_bass_kernel_spmd(nc, in_maps, core_ids=list(range(B)))
    out = np.stack([res.results[c]["y"].reshape(S, D) for c in range(B)])
    return out.astype(np.float32)


# revision 4
# speedup vs baseline: 2.2656x; 1.6069x over previous
"""MultiHeadedAttention Trainium2 Bass kernel.

Reference (per batch element b, full shapes B=8, S=1024, D=512, H=8, DK=64):
    Q = x_q @ Wq + bq ; K = x_k @ Wk + bk ; V = x_v @ Wv + bv   (per-head split)
    S = Q K^T / sqrt(DK);  S masked where mask==0 -> -inf
    P = softmax(S); P zeroed where mask==0
    Y = (P V, heads concat) @ Wo + bo

Sharding: pure data parallel over batch — core c computes batch element c.
No collectives. Host transposes x inputs so the kernel needs no on-chip
input transposes, precomputes the additive exp-space mask bias, and
rounds matmul operands to bf16 (PSUM accumulation stays fp32; measured
bf16 matmul streams ~15% faster than f32r on HW and halves HBM+SBUF).

Engine assignment (HW shows ~1us fixed cost per DVE op, so DVE glue is
offloaded): ACT does exp + all PSUM->SBUF moves (Identity/Copy and Exp
live in the same activation table, so no table swaps); Pool (gpsimd)
broadcasts the softmax reciprocal across partitions; DVE keeps only the
reciprocal and the normalize multiply.

All tile pools and tiles are allocated ONCE, outside the benchmark
loop: iterations then overlap through per-tile dependencies instead of
a pool-lifetime barrier, so the next iteration's input DMA streams in
under the current iteration's attention phase. Input DMAs issue on
SP's queue; y stores issue on ACT's queue (separate FIFO, so the next
iteration's loads don't queue behind this iteration's stores).

Per-core layout (bf16 matmul operands; PSUM accumulates f32):
  xT        [in=512, S]  (host-transposed)
  QT, KT    [feat, S]   psum[out128, q512] += Wq[in128, out128].T @ xT[in128, q512]
                        bias folded into the ACT Identity that moves PSUM->SBUF
  V natural [S, feat]   psum[row128, f512] += xT_v[in128, row128].T @ Wv[in128, f512]
                        (+ ones-row x bv outer product), stored interleaved as
                        v_aug[row128, head, 65] with a ones column per head
                        (softmax denominator for free); one ACT copy per row tile
  S^T       [k128, q512] = KT_h[d64, k128].T @ QT_h[d64, q512]
                        head pairs packed into PE row groups 0/64 via
                        tile_position
  P^T       = Exp(S^T/8 + maskbias_k)      (ACT, one call per [128,1024])
  (PV)^T+den[65, q512]  += v_aug_h[k128, 65].T @ P^T[k128, q512]  (row 64 = denom)
  norm      rec = 1/den (DVE), rbs = bcast rec over 64 partitions (Pool),
            at_pair[t][h%2*64 :+64, q] = (PV)^T * rbs (DVE; write may shift
            base partition by 64 for odd heads -> K=128 below)
  Y natural [q128, 512] += at_pair[t][:, q128].T @ Wo[feat128, out512] (+ bo)

PSUM (8 banks): mm512 tag (proj + out-proj, [128,512]) x2, scores
[128,1024] x2 = 4 banks, PV/denominator [128,512] x2.
"""

import numpy as np

B, S, D, H = 8, 1024, 512, 8
DK = D // H  # 64
P = 128
KI = D // P  # 4 in-feature tiles
RT = S // P  # 8 row tiles
QC = S // 512  # 2 q chunks of 512
HP = H // 2  # 4 head pairs
MASK_NEG = -30000.0  # exp(-30000) == 0.0 in f32

_CACHED = {}


def _build_nc(loop_reps=None):
    import concourse.mybir as mybir
    import concourse.tile as tile
    from concourse import bacc

    f32 = mybir.dt.float32
    bf16 = mybir.dt.bfloat16
    EXP = mybir.ActivationFunctionType.Exp
    CPY = mybir.ActivationFunctionType.Copy
    IDN = mybir.ActivationFunctionType.Identity
    ISCALE = 1.0 / float(np.sqrt(DK))

    nc = bacc.Bacc("TRN2")

    xqT_d = nc.dram_tensor("xqT", (KI, P, S), bf16, kind="ExternalInput")
    xkT_d = nc.dram_tensor("xkT", (KI, P, S), bf16, kind="ExternalInput")
    xvT_d = nc.dram_tensor("xvT", (KI, P, S), bf16, kind="ExternalInput")
    maskb_d = nc.dram_tensor("maskb", (P, RT), f32, kind="ExternalInput")
    wq_d = nc.dram_tensor("wq", (KI, P, D), bf16, kind="ExternalInput")
    wk_d = nc.dram_tensor("wk", (KI, P, D), bf16, kind="ExternalInput")
    wv_d = nc.dram_tensor("wv", (KI, P, D), bf16, kind="ExternalInput")
    wo_d = nc.dram_tensor("wo", (KI, P, D), bf16, kind="ExternalInput")
    bq_d = nc.dram_tensor("bq", (P, KI), f32, kind="ExternalInput")
    bk_d = nc.dram_tensor("bk", (P, KI), f32, kind="ExternalInput")
    bv_d = nc.dram_tensor("bv", (1, D), bf16, kind="ExternalInput")
    bo_d = nc.dram_tensor("bo", (1, D), bf16, kind="ExternalInput")
    y_d = nc.dram_tensor("y", (RT, P, D), f32, kind="ExternalOutput")

    with tile.TileContext(nc) as tc, nc.allow_low_precision(
        reason="bf16 matmul operands; accumulation stays fp32 in PSUM"
    ):
        from contextlib import ExitStack

        with ExitStack() as ctx:
            const = ctx.enter_context(tc.tile_pool(name="const", bufs=1))
            persist = ctx.enter_context(tc.tile_pool(name="persist", bufs=1))
            xt_pool = ctx.enter_context(tc.tile_pool(name="xt", bufs=1))
            pt_pool = ctx.enter_context(tc.tile_pool(name="pt", bufs=24))
            rec_pool = ctx.enter_context(tc.tile_pool(name="rec", bufs=4))
            rbs_pool = ctx.enter_context(tc.tile_pool(name="rbs", bufs=4))
            y_pool = ctx.enter_context(tc.tile_pool(name="y", bufs=3))
            mm_ps = ctx.enter_context(tc.tile_pool(name="mmps", bufs=2, space="PSUM"))
            at_ps = ctx.enter_context(tc.tile_pool(name="spsum", bufs=2, space="PSUM"))
            ov_ps = ctx.enter_context(tc.tile_pool(name="opsum", bufs=2, space="PSUM"))

            wq = [const.tile([P, D], bf16, name=f"wq{i}", tag=f"wq{i}") for i in range(KI)]
            wk = [const.tile([P, D], bf16, name=f"wk{i}", tag=f"wk{i}") for i in range(KI)]
            wv = [const.tile([P, D], bf16, name=f"wv{i}", tag=f"wv{i}") for i in range(KI)]
            wo = [const.tile([P, D], bf16, name=f"wo{i}", tag=f"wo{i}") for i in range(KI)]
            bq_t = const.tile([P, KI], f32, name="bq_t", tag="bq")
            bk_t = const.tile([P, KI], f32, name="bk_t", tag="bk")
            bv_t = const.tile([1, D], bf16, name="bv_t", tag="bv")
            bo_t = const.tile([1, D], bf16, name="bo_t", tag="bo")
            maskb = const.tile([P, RT], f32, name="maskb", tag="maskb")
            ones_t = const.tile([1, P], bf16, name="ones_t", tag="ones")
            nc.gpsimd.memset(ones_t[:], 1.0)

            # persistent intermediates
            qt = [persist.tile([P, S], bf16, name=f"qt{i}", tag=f"qt{i}") for i in range(KI)]
            kt_ = [persist.tile([P, S], bf16, name=f"kt{i}", tag=f"kt{i}") for i in range(KI)]
            v_aug = [persist.tile([P, H, DK + 1], bf16, name=f"va{i}", tag=f"va{i}") for i in range(RT)]
            # head-pair attention outputs: pair t rows 0:64 = head 2t,
            # rows 64:128 = head 2t+1 => feature rows 128t..128t+127
            at = [persist.tile([P, S], bf16, name=f"at{i}", tag=f"at{i}") for i in range(HP)]
            xqT = [xt_pool.tile([P, S], bf16, name=f"xq{i}", tag=f"xq{i}") for i in range(KI)]
            xkT = [xt_pool.tile([P, S], bf16, name=f"xk{i}", tag=f"xk{i}") for i in range(KI)]
            xvT = [xt_pool.tile([P, S], bf16, name=f"xv{i}", tag=f"xv{i}") for i in range(KI)]

            # ones columns of v_aug are never overwritten by the loop body
            # (the V copy writes [:, :, 0:DK] only), so set them once.
            for rt in range(RT):
                nc.gpsimd.memset(v_aug[rt][:, :, DK : DK + 1], 1.0)

            def emit():
                # DMA in consumption order (queue is FIFO): q-path first so
                # the first projection can start after ~1.5MB, then k-path,
                # v-path, output weights. All input loads on SP's queue —
                # the next iteration's loads start as soon as each target
                # tile's last read of this iteration retires.
                for i in range(KI):
                    nc.sync.dma_start(wq[i][:], wq_d[i])
                    nc.sync.dma_start(xqT[i][:], xqT_d[i])
                nc.sync.dma_start(bq_t[:], bq_d[:])
                for i in range(KI):
                    nc.sync.dma_start(wk[i][:], wk_d[i])
                    nc.sync.dma_start(xkT[i][:], xkT_d[i])
                nc.sync.dma_start(bk_t[:], bk_d[:])
                nc.sync.dma_start(maskb[:], maskb_d[:])
                for i in range(KI):
                    nc.sync.dma_start(wv[i][:], wv_d[i])
                    nc.sync.dma_start(xvT[i][:], xvT_d[i])
                nc.sync.dma_start(bv_t[:], bv_d[:])
                for i in range(KI):
                    nc.sync.dma_start(wo[i][:], wo_d[i])
                nc.sync.dma_start(bo_t[:], bo_d[:])

                # QT / KT projections; ACT moves PSUM->SBUF, adding the
                # per-partition bias during the copy.
                for w, x, bias, dst in ((wq, xqT, bq_t, qt), (wk, xkT, bk_t, kt_)):
                    for o in range(KI):
                        for qc in range(QC):
                            ps = mm_ps.tile([P, 512], f32, name="psA", tag="psA")
                            for ki in range(KI):
                                nc.tensor.matmul(
                                    ps[:],
                                    w[ki][:, o * P : (o + 1) * P],
                                    x[ki][:, qc * 512 : (qc + 1) * 512],
                                    start=(ki == 0),
                                    stop=(ki == KI - 1),
                                )
                            nc.scalar.activation(
                                dst[o][:, qc * 512 : (qc + 1) * 512],
                                ps[:],
                                IDN,
                                bias=bias[:, o : o + 1],
                            )

                # V natural -> v_aug (interleaved heads; ones columns are
                # set once outside the loop)
                for rt in range(RT):
                    ps = mm_ps.tile([P, 512], f32, name="psA", tag="psA")
                    for ki in range(KI):
                        nc.tensor.matmul(
                            ps[:],
                            xvT[ki][:, rt * P : (rt + 1) * P],
                            wv[ki][:],
                            start=(ki == 0),
                            stop=False,
                        )
                    nc.tensor.matmul(
                        ps[:],
                        ones_t[0:1, 0:P],
                        bv_t[0:1, :],
                        start=False,
                        stop=True,
                    )
                    nc.scalar.activation(
                        v_aug[rt][:, :, 0:DK],
                        ps[:].rearrange("p (h d) -> p h d", h=H),
                        CPY,
                    )

                # --- attention, one head pair at a time ---
                for t in range(HP):
                    pts = [
                        [pt_pool.tile([P, S], bf16, name="pt", tag="pt") for _ in range(RT)]
                        for _ in range(2)
                    ]
                    # sub 0's PV chains consume pt tiles in lockstep with
                    # the exp stream so half the pool frees at pair end
                    # (the next pair's exps then aren't slot-starved).
                    ops00 = ov_ps.tile([P, 512], f32, name="ops", tag="ops")
                    ops01 = ov_ps.tile([P, 512], f32, name="ops", tag="ops")
                    for kt in range(RT):
                        for sub in range(2):
                            off = sub * DK
                            sps = at_ps.tile([P, S], f32, name="sps", tag="sps")
                            for qc in range(QC):
                                nc.tensor.matmul(
                                    sps[:, qc * 512 : (qc + 1) * 512],
                                    kt_[t][off : off + DK, kt * P : (kt + 1) * P],
                                    qt[t][off : off + DK, qc * 512 : (qc + 1) * 512],
                                    start=True,
                                    stop=True,
                                    tile_position=(off, 0),
                                )
                            nc.scalar.activation(
                                pts[sub][kt][:],
                                sps[:],
                                EXP,
                                bias=maskb[:, kt : kt + 1],
                                scale=ISCALE,
                            )
                        for qc, ops in ((0, ops00), (1, ops01)):
                            nc.tensor.matmul(
                                ops[0 : DK + 1, :],
                                v_aug[kt][:, 2 * t, 0 : DK + 1],
                                pts[0][kt][:, qc * 512 : (qc + 1) * 512],
                                start=(kt == 0),
                                stop=(kt == RT - 1),
                            )
                    for sub in range(2):
                        h = 2 * t + sub
                        off = sub * DK
                        for qc in range(QC):
                            if sub == 0:
                                ops = ops00 if qc == 0 else ops01
                            else:
                                ops = ov_ps.tile(
                                    [P, 512], f32, name="ops", tag="ops"
                                )
                                for kt in range(RT):
                                    nc.tensor.matmul(
                                        ops[0 : DK + 1, :],
                                        v_aug[kt][:, h, 0 : DK + 1],
                                        pts[sub][kt][:, qc * 512 : (qc + 1) * 512],
                                        start=(kt == 0),
                                        stop=(kt == RT - 1),
                                    )
                            rec = rec_pool.tile([1, 512], f32, name="rec", tag="rec")
                            nc.vector.reciprocal(rec[0:1, :], ops[DK : DK + 1, :])
                            rbs = rbs_pool.tile([DK, 512], f32, name="rbs", tag="rbs")
                            nc.gpsimd.partition_broadcast(rbs[:], rec[0:1, :])
                            nc.vector.tensor_mul(
                                at[t][off : off + DK, qc * 512 : (qc + 1) * 512],
                                ops[0:DK, :],
                                rbs[:],
                            )

                # --- output projection: contraction K=128 over head pairs ---
                # y stores issue on ACT's queue: ACT produced yt right before,
                # and the SP queue stays clear for the next iteration's loads.
                for rt in range(RT):
                    yps = mm_ps.tile([P, D], f32, name="psA", tag="psA")
                    for t in range(HP):
                        nc.tensor.matmul(
                            yps[:],
                            at[t][:, rt * P : (rt + 1) * P],
                            wo[t][:],
                            start=(t == 0),
                            stop=False,
                        )
                    nc.tensor.matmul(
                        yps[:],
                        ones_t[0:1, 0:P],
                        bo_t[0:1, :],
                        start=False,
                        stop=True,
                    )
                    yt = y_pool.tile([P, D], f32, name="yt", tag="yt")
                    nc.scalar.activation(yt[:], yps[:], CPY)
                    nc.scalar.dma_start(y_d[rt], yt[:])

            if loop_reps is None:
                emit()
            else:
                # benchmark variant: repeat the whole body on-device
                ET = mybir.EngineType
                with tc.For_i(
                    0,
                    loop_reps,
                    1,
                    hint_engines=(ET.PE, ET.Activation, ET.DVE, ET.SP, ET.Pool),
                ):
                    emit()

    nc.compile()
    return nc


def get_nc(loop_reps=None):
    key = ("nc", loop_reps)
    if key not in _CACHED:
        _CACHED[key] = _build_nc(loop_reps)
    return _CACHED[key]


def make_in_maps(query, key, value, mask, Wq, bq, Wk, bk, Wv, bv, Wo, bo):
    """Shard full inputs into per-core input maps (host-side numpy)."""
    import ml_dtypes

    f = np.float32
    bf = ml_dtypes.bfloat16
    query = np.asarray(query, f)
    key = np.asarray(key, f)
    value = np.asarray(value, f)
    mask = np.asarray(mask)

    def wtiles(W):
        return np.ascontiguousarray(
            np.asarray(W, f).reshape(KI, P, D).astype(bf)
        )

    wq_t, wk_t, wv_t, wo_t = wtiles(Wq), wtiles(Wk), wtiles(Wv), wtiles(Wo)
    bq_t = np.ascontiguousarray(np.asarray(bq, f).reshape(KI, P).T)
    bk_t = np.ascontiguousarray(np.asarray(bk, f).reshape(KI, P).T)
    bv_t = np.ascontiguousarray(np.asarray(bv, f).reshape(1, D).astype(bf))
    bo_t = np.ascontiguousarray(np.asarray(bo, f).reshape(1, D).astype(bf))

    in_maps = []
    for c in range(B):
        xqT = np.ascontiguousarray(query[c].T).reshape(KI, P, S).astype(bf)
        xkT = np.ascontiguousarray(key[c].T).reshape(KI, P, S).astype(bf)
        xvT = np.ascontiguousarray(value[c].T).reshape(KI, P, S).astype(bf)
        mb = np.where(mask[c, 0] == 0, f(MASK_NEG), f(0.0)).astype(f)
        mb = np.ascontiguousarray(mb.reshape(RT, P).T)
        in_maps.append(
            {
                "xqT": xqT,
                "xkT": xkT,
                "xvT": xvT,
                "maskb": mb,
                "wq": wq_t,
                "wk": wk_t,
                "wv": wv_t,
                "wo": wo_t,
                "bq": bq_t,
                "bk": bk_t,
                "bv": bv_t,
                "bo": bo_t,
            }
        )
    return in_maps


def kernel(**inputs):
    from concourse.bass_utils import run_bass_kernel_spmd

    nc = get_nc()
    in_maps = make_in_maps(**inputs)
    res = run_bass_kernel_spmd(nc, in_maps, core_ids=list(range(B)))
    out = np.stack([res.results[c]["y"].reshape(S, D) for c in range(B)])
    return out.astype(np.float32)
